# revision 65
# baseline (speedup 1.0000x reference)
"""TRN2 Bass kernel for nn_LinearAttention (B=4, L=4096, D=1024, H=16, dh=64).

Sharding: core c = batch c//2, heads (c%2)*8..+8. Zero cross-core comms; the
two half-head partial output projections per batch are summed on the host.

Per-core pipeline (super-tiles of 512 tokens, scan chunks of 128):
  1. QKV projection:
     - q/k: fp8e4 DoubleRow matmuls (0.5 cyc/row, K=256 per call). Weights are
       scaled x32 on the host (plain w_qkv values sit in e4m3's subnormal
       range and quantize at 6-50% error; x32 moves them to normals). A bias
       ones-row (weight 32) makes PSUM hold 32(z+1); phi is exact in 2 passes
       via 32*phi(z) = max(min(32*e^z, 32), 32z+32):
       Act Exp(scale=1/32, bias=ln32-1) then one DVE scalar_tensor_tensor.
       The x32 on q/k cancels between numerator and denominator.
     - v: fp8 DoubleRow, 2-term compensated (x_hi(w_hi+w_lo)); the very first
       chunk adds the x_lo w_hi term (early tokens average too few v's to
       absorb x-quantization noise). The copy to bf16 applies 1/128.
  2. Chunked linear attention, bf16 operands / f32 PSUM:
     - k chunks go token-major via XBAR DMA-transpose (sync queue, 16x128
       tiles at 14ns) - the PE never transposes.
     - A^T = K^T Q for all 8 heads into one 2-bank PSUM tile (parity-split
       tile rows) -> masked-copy per parity half (DVE, doubles as the
       PSUM->SBUF move) -> den matmuls first (rden overlaps O) -> O = A_m V +
       Q S8. den > 0 always, so the reference eps-clamp is skipped and
       rden = 1/den directly; every scale factor is folded into w2 on the
       host. State [S | z/32] lives PERSISTENTLY in one PSUM bank (start only
       at chunk 0) with a per-chunk bf16 snapshot for the next chunk's
       Q S / q.z reads.
  3. Normalized o (bf16, token-major) is XBAR DMA-transposed to feature-major;
     the idle Pool engine (all-SBUF, so legal there) derives an fp8 hi/lo
     split, and the output projection runs as 3-term compensated fp8
     DoubleRow (o_hi w_hi + o_hi w_lo + o_lo w_hi, weights x32 fp8 hi/lo on
     the host) at 0.75x the bf16 cost. bf16 partials are DMA'd out per chunk;
     the host upcasts and sums core pairs in f32.

Schedule: emission order is the engine execution order, so it is tuned as a
software pipeline: per chunk iteration of super-tile s we emit [attention
chunk] [output projection of the chunk 4 back] [2-4 q/k projections of s+1,
with k-transposes as each k group completes] [one v projection of s+1]. The
output projection lags 4 chunks so its fp8 operand chain (XBAR transpose ->
Pool hi/lo split) never blocks the PE. A junk-matmul warmup burst at t=0
ramps the PE p-state inside the initial DMA shadow. Engine placement:
PE matmuls only; Act: exp, v/sz8/o3sb PSUM->SBUF moves; DVE: phi combine,
mask-multiplies, reciprocal, o normalize; Pool: fp8 o split + weight DMAs;
sync: x loads, XBAR transposes, output stores. PSUM banks: 3 projection +
2 A^T + 1 O + 1 state + 1 den = 8.
"""
import sys

sys.path.insert(0, "/opt/trn_rl_repo")
import numpy as np
import ml_dtypes

F8 = ml_dtypes.float8_e4m3
BFNP = ml_dtypes.bfloat16

D = 1024
L = 4096
B = 4
H = 16
DH = 64
FPC = 512          # features per core (8 heads x 64)
C = 128            # scan chunk
ST = 512           # super-tile tokens
NCH = ST // C      # 4
NST = L // ST      # 8
EPS = 1e-6
V3TERM_ST = 1      # super-tiles using 3-term compensated v (rest use 2-term)

_CACHE = {}


def _build_nc():
    import concourse.bacc as bacc
    import concourse.mybir as mybir
    import concourse.tile as tile

    dt = mybir.dt
    f32, fp8, bf16 = dt.float32, dt.float8e4, dt.bfloat16
    Alu = mybir.AluOpType
    Act = mybir.ActivationFunctionType
    PM = mybir.MatmulPerfMode

    nc = bacc.Bacc("TRN2", target_bir_lowering=False, debug=True)

    xhi_d = nc.dram_tensor("xhi", [128, 4, 2, L], fp8, kind="ExternalInput")
    xlo_d = nc.dram_tensor("xlo", [128, 4, 2, L], fp8, kind="ExternalInput")
    wqk_d = nc.dram_tensor("wqk", [128, 8, 4, 2, 128], fp8, kind="ExternalInput")
    wvh_d = nc.dram_tensor("wvh", [128, 4, 2, FPC], fp8, kind="ExternalInput")
    wvl_d = nc.dram_tensor("wvl", [128, 4, 2, FPC], fp8, kind="ExternalInput")
    w28h_d = nc.dram_tensor("w28h", [128, 2, 2, 8, 128], fp8, kind="ExternalInput")
    w28l_d = nc.dram_tensor("w28l", [128, 2, 2, 8, 128], fp8, kind="ExternalInput")
    maskT_d = nc.dram_tensor("maskT", [C, C], f32, kind="ExternalInput")
    # [st, c, p, g, t]: output feature 128*g+p at token 512*st+128*c+t
    outC_d = nc.dram_tensor(
        "outC", [NST, NCH, 128, 8, C], bf16, kind="ExternalOutput"
    )

    from contextlib import ExitStack

    with tile.TileContext(nc) as tc, ExitStack() as es:
        if True:
            wp = es.enter_context(tc.tile_pool(name="wp", bufs=1))
            xp = es.enter_context(tc.tile_pool(name="xp", bufs=4))
            qp = es.enter_context(tc.tile_pool(name="qp", bufs=6))
            scr = es.enter_context(tc.tile_pool(name="scr", bufs=6))
            vp = es.enter_context(tc.tile_pool(name="vp", bufs=2))
            kp = es.enter_context(tc.tile_pool(name="kp", bufs=9))
            atsp = es.enter_context(tc.tile_pool(name="atsp", bufs=5))
            szsp = es.enter_context(tc.tile_pool(name="szsp", bufs=5))
            dp = es.enter_context(tc.tile_pool(name="dp", bufs=8))
            ocp = es.enter_context(tc.tile_pool(name="ocp", bufs=5))
            osp = es.enter_context(tc.tile_pool(name="osp", bufs=3))
            o8p = es.enter_context(tc.tile_pool(name="o8p", bufs=3))
            o3p = es.enter_context(tc.tile_pool(name="o3p", bufs=2))
            projp = es.enter_context(tc.tile_pool(name="projp", bufs=3, space="PSUM"))
            atp = es.enter_context(tc.tile_pool(name="atp", bufs=1, space="PSUM"))
            obp = es.enter_context(tc.tile_pool(name="obp", bufs=1, space="PSUM"))
            szp = es.enter_context(tc.tile_pool(name="szp", bufs=1, space="PSUM"))
            denp = es.enter_context(tc.tile_pool(name="denp", bufs=1, space="PSUM"))

            # ---- resident constants / weights (wqk split so fc0 can start
            # as soon as the first half lands) ----
            wqk4 = []
            for q4 in range(4):
                wq = wp.tile([128, 2, 4, 2, 128], fp8, tag=f"wqk{q4}")
                nc.scalar.dma_start(wq[:], wqk_d[:, 2 * q4 : 2 * q4 + 2])
                wqk4.append(wq)
            wvh = wp.tile([128, 4, 2, FPC], fp8, tag="wvh")
            nc.gpsimd.dma_start(wvh[:], wvh_d[:])
            wvl = wp.tile([128, 4, 2, FPC], fp8, tag="wvl")
            nc.gpsimd.dma_start(wvl[:], wvl_d[:])
            maskT = wp.tile([C, C], f32, tag="maskT")
            nc.gpsimd.dma_start(maskT[:], maskT_d[:])
            w28h = wp.tile([128, 2, 2, 8, 128], fp8, tag="w28h")
            w28l = wp.tile([128, 2, 2, 8, 128], fp8, tag="w28l")
            nc.gpsimd.dma_start(w28h[:], w28h_d[:])
            nc.gpsimd.dma_start(w28l[:], w28l_d[:])
            wb8 = wp.tile([128, 2, 128], fp8, tag="wb8")
            nc.vector.memset(wb8[:].bitcast(f32), 0.0)
            nc.vector.memset(wb8[0:1, 0, :], 32.0)
            xb8 = wp.tile([128, 2, ST], fp8, tag="xb8")
            nc.vector.memset(xb8[:].bitcast(f32), 0.0)
            nc.vector.memset(xb8[0:1, 0, :], 1.0)
            ones32 = wp.tile([128, 1], bf16, tag="ones32")
            nc.vector.memset(ones32[:], 1.0 / 32.0)
            bm1 = wp.tile([128, 1], f32, tag="bm1")
            nc.vector.memset(bm1[:], float(np.log(32.0)))

            # PE clock-ramp warmup: the PE would idle ~3.5us waiting for the
            # first weight/x DMAs anyway; junk matmuls during that window ramp
            # the p-state so real work starts at full clock.
            warm = projp.tile([128, ST], f32, tag="proj", name="warmup")
            for w in range(8):
                nc.tensor.matmul(
                    warm[:],
                    wb8[:],
                    xb8[:, :, :],
                    start=(w == 0),
                    stop=(w == 7),
                    perf_mode=PM.DoubleRow,
                    skip_group_check=True,
                )

            # persistent scan state [S | z/32] per head pair, in PSUM.
            # Full-bank tile (2048B row pitch) so pending-zero bookkeeping
            # stays row-aligned for base_partition=64 matmul outputs.
            Sz_t = szp.tile([128, 512], f32, tag="Sz")
            Sz = Sz_t[:, 0:260].rearrange("p (j e) -> p j e", e=DH + 1)
            sz8_init = szsp.tile([128, 4, DH + 1], bf16, tag="Sz8", name="sz8i")
            nc.vector.memset(sz8_init[:], 0.0)
            szh = [sz8_init]  # rolling snapshot holder

            def emit_proj_tiles(st):
                t0 = st * ST
                xhi = xp.tile([128, 4, 2, ST], fp8, tag="xhi", name=f"xhi{st}")
                nc.sync.dma_start(xhi[:], xhi_d[:, :, :, t0 : t0 + ST])
                T = dict(xhi=xhi)
                if st < V3TERM_ST:
                    xlo = xp.tile([128, 4, 2, ST], fp8, tag="xlo", name=f"xlo{st}")
                    nc.sync.dma_start(xlo[:], xlo_d[:, :, :, t0 : t0 + ST])
                    T["xlo"] = xlo
                T["qTb"] = qp.tile([128, 4, ST], bf16, tag="qTb", name=f"qTb{st}")
                T["kTb"] = qp.tile([128, 4, ST], bf16, tag="kTb", name=f"kTb{st}")
                T["v1"] = vp.tile([128, NCH, 8, DH], bf16, tag="v1", name=f"v1{st}")
                return T

            def emit_fc(T, fc):
                pq = projp.tile([128, ST], f32, tag="proj", name=f"pq_{fc}")
                for hh in range(2):
                    sl = slice(256 * hh, 256 * hh + 256)
                    for j in range(4):
                        nc.tensor.matmul(
                            pq[:, sl],
                            wqk4[fc // 2][:, fc % 2, j, :, :],
                            T["xhi"][:, j, :, sl],
                            start=(hh == 0 and j == 0),
                            stop=(hh == 1 and j == 3),
                            perf_mode=PM.DoubleRow,
                            skip_group_check=True,
                        )
                # PSUM holds 32z (no bias row):
                # 32*phi(z) = min(32*e^z, max(32z, 0) + 32); the exp branch
                # (Act) and linear branch (DVE) race, the all-SBUF min runs
                # on the otherwise-idle Pool engine.
                texp = scr.tile([128, ST], bf16, tag="texp", name=f"texp{fc}")
                nc.scalar.activation(
                    texp[:], pq[:], Act.Exp, bias=bm1[:], scale=1.0 / 32.0
                )
                lin = scr.tile([128, ST], bf16, tag="lin", name=f"lin{fc}")
                nc.vector.tensor_scalar(
                    lin[:], pq[:], 0.0, 32.0, Alu.max, Alu.add
                )
                dst = T["qTb"][:, fc, :] if fc < 4 else T["kTb"][:, fc - 4, :]
                nc.vector.tensor_tensor(dst, texp[:], lin[:], Alu.min)

            def emit_v(T, tcc):
                pv = projp.tile([128, FPC], f32, tag="proj", name=f"pv{tcc}")
                xc_hi = T["xhi"][:, :, :, tcc * 128 : (tcc + 1) * 128]
                if "xlo" in T and tcc == 0:
                    xc_lo = T["xlo"][:, :, :, tcc * 128 : (tcc + 1) * 128]
                    terms = ((xc_hi, wvh), (xc_hi, wvl), (xc_lo, wvh))
                else:
                    terms = ((xc_hi, wvh), (xc_hi, wvl))
                ncalls = 4 * len(terms)
                n = 0
                for hh in range(2):
                    sl = slice(256 * hh, 256 * hh + 256)
                    for xa, wv in terms:
                        for j in range(4):
                            n += 1
                            nc.tensor.matmul(
                                pv[:, sl],
                                xa[:, j, :, :],
                                wv[:, j, :, sl],
                                start=(n == 1 or n == ncalls + 1),
                                stop=(n == ncalls or n == 2 * ncalls),
                                perf_mode=PM.DoubleRow,
                                skip_group_check=True,
                            )
                nc.scalar.mul(
                    T["v1"][:, tcc, :, :],
                    pv[:].rearrange("p (h e) -> p h e", e=DH),
                    1.0 / 128.0,
                )

            def alloc_ksb(ksbs, st):
                for tcc in range(NCH):
                    ksbs[(st, tcc)] = kp.tile(
                        [128, 8, DH], bf16, tag="ksb", name=f"ksb{st * NCH + tcc}"
                    )

            def emit_ktr_j(T, ksbs, st, j):
                # XBAR DMA-transpose k group j to token-major for every chunk:
                # [128 feats, 128 toks] -> ksb[:, 2j:2j+2, :] (= heads 2j,2j+1).
                for tcc in range(NCH):
                    tsl = slice(tcc * 128, (tcc + 1) * 128)
                    nc.sync.dma_start_transpose(
                        ksbs[(st, tcc)][:, 2 * j : 2 * j + 2, :],
                        T["kTb"][:, j, tsl],
                    )

            def emit_chunk_a(T, st, tcc):
                # A^T + masked-copy; the PE cover work for the DVE mask is
                # emitted by the driver between chunk_a and chunk_b.
                cg = st * NCH + tcc
                tsl = slice(tcc * 128, (tcc + 1) * 128)
                qTb, kTb, v1 = T["qTb"], T["kTb"], T["v1"]

                # -- A^T = K^T Q; bank g = heads of parity g (a PE tile-row
                # config may not change within a PSUM bank); one 2-bank tile
                # so the masked-copy is a single DVE op --
                ATp = atp.tile([128, 2, 4, C], f32, tag="ATp", name=f"atp{cg}")
                for h in range(8):
                    po = 64 * (h % 2)
                    fq = h // 2
                    nc.tensor.matmul(
                        ATp[:, h % 2, h // 2, :],
                        kTb[po : po + 64, fq, tsl],
                        qTb[po : po + 64, fq, tsl],
                        start=(h <= 1),
                        stop=(h >= 6),
                        skip_group_check=True,
                    )
                ATs = atsp.tile([128, 8, C], bf16, tag="ATs", name=f"ats{cg}")
                for g in range(2):
                    nc.vector.tensor_tensor(
                        ATs[:, 4 * g : 4 * g + 4, :],
                        ATp[:, g],
                        maskT[:].unsqueeze(1).broadcast_to([C, 4, C]),
                        Alu.mult,
                    )

                return ATs

            def emit_chunk_b(T, st, tcc, outT_sb, ohi, olo, ATs, sz8_prev, ksb):
                cg = st * NCH + tcc
                tsl = slice(tcc * 128, (tcc + 1) * 128)
                qTb, v1 = T["qTb"], T["v1"]
                den = denp.tile([128, 8], f32, tag="den", name=f"den{cg}")

                HORD = (0, 2, 4, 6, 1, 3, 5, 7)  # even parity first: its
                # masked half lands first, so den/O can start sooner

                # -- den = A_m^T ones/32 + q . z8 (before O: rden overlaps O) --
                for i, h in enumerate(HORD):
                    po = 64 * (h % 2)
                    fq = h // 2
                    nc.tensor.matmul(
                        den[:, h : h + 1],
                        ATs[:, 4 * (h % 2) + h // 2, :],
                        ones32[:],
                        start=(i == 0),
                        stop=False,
                        skip_group_check=True,
                    )
                    nc.tensor.matmul(
                        den[:, h : h + 1],
                        qTb[po : po + 64, fq, tsl],
                        sz8_prev[po : po + 64, h // 2, DH : DH + 1],
                        start=False,
                        stop=(i == 7),
                        skip_group_check=True,
                    )
                # den > 0 always (phi > 0), so the reference eps-clamp is a
                # numeric no-op; the *8 scale restore is folded into w2.
                rden = dp.tile([C, 8], f32, tag="rden", name=f"rden{cg}")
                nc.vector.reciprocal(rden[:], den[:])

                # -- O = A_m V + Q S8 (one bank, 8 heads) --
                Ob = obp.tile([128, 8, DH], f32, tag="Ob", name=f"ob{cg}")
                for i, h in enumerate(HORD):
                    po = 64 * (h % 2)
                    fq = h // 2
                    nc.tensor.matmul(
                        Ob[:, h, :],
                        ATs[:, 4 * (h % 2) + h // 2, :],
                        v1[:, tcc, h, :],
                        start=(i == 0),
                        stop=False,
                        skip_group_check=True,
                    )
                    nc.tensor.matmul(
                        Ob[:, h, :],
                        qTb[po : po + 64, fq, tsl],
                        sz8_prev[po : po + 64, h // 2, 0:DH],
                        start=False,
                        stop=(i == 7),
                        skip_group_check=True,
                    )

                # -- state update into the persistent Sz bank --
                for h in range(8):
                    po = 64 * (h % 2)
                    j = h // 2
                    nc.tensor.matmul(
                        Sz[po : po + 64, j, 0:DH],
                        ksb[:, h, :],
                        v1[:, tcc, h, :],
                        start=(cg == 0 and h <= 1),
                        stop=False,
                        skip_group_check=True,
                    )
                    nc.tensor.matmul(
                        Sz[po : po + 64, j, DH : DH + 1],
                        ksb[:, h, :],
                        ones32[:],
                        start=False,
                        stop=(cg == NST * NCH - 1 and h == 7),
                        skip_group_check=True,
                    )
                sz8_new = szsp.tile(
                    [128, 4, DH + 1], bf16, tag="Sz8", name=f"sz8_{cg}"
                )
                nc.scalar.copy(sz8_new[:], Sz[:])
                szh[0] = sz8_new

                # -- normalize -> bf16 o_c (token-major) --
                oc8 = ocp.tile([C, 8, DH], bf16, tag="oc8", name=f"oc8{cg}")
                nc.vector.tensor_tensor(
                    oc8[:],
                    Ob[:],
                    rden[:].unsqueeze(2).broadcast_to([C, 8, DH]),
                    Alu.mult,
                )

                # -- XBAR DMA-transpose o_c to feature-major --
                oc_flat = oc8[:].rearrange("p h e -> p (h e)")
                for g in range(4):
                    nc.sync.dma_start_transpose(
                        outT_sb[:, g, tsl], oc_flat[:, g * 128 : (g + 1) * 128]
                    )
                # fp8 hi/lo split of feature-major o, on the (idle) Pool
                # engine: all-SBUF so it is legal there
                nc.gpsimd.tensor_copy(ohi[:, :, tsl], outT_sb[:, :, tsl])
                nc.gpsimd.tensor_tensor(
                    olo[:, :, tsl],
                    outT_sb[:, :, tsl],
                    ohi[:, :, tsl],
                    Alu.subtract,
                )

            def emit_outproj_chunk(st, ohi, olo, o3sb, tcc):
                # project one 128-token chunk to all 1024 output features:
                # 3-term compensated fp8 DoubleRow (o_hi w_hi + o_hi w_lo +
                # o_lo w_hi), then store (>=1KB contiguous runs both sides)
                tsl = slice(tcc * 128, (tcc + 1) * 128)
                last = st == NST - 1 and tcc == NCH - 1
                for half in range(2):
                    po3 = projp.tile(
                        [128, 4, C], f32, tag="proj", name=f"po3_{st}_{tcc}_{half}"
                    )
                    for g4 in range(4):
                        oc = 4 * half + g4
                        n = 0
                        for oa, wv in ((ohi, w28h), (ohi, w28l), (olo, w28h)):
                            for u in range(2):
                                n += 1
                                nc.tensor.matmul(
                                    po3[:, g4, :],
                                    wv[:, u, :, oc, :],
                                    oa[:, 2 * u : 2 * u + 2, tsl],
                                    start=(n == 1),
                                    stop=(n == 6),
                                    perf_mode=PM.DoubleRow,
                                    skip_group_check=True,
                                )
                    dst = o3sb[:, tcc, 4 * half : 4 * half + 4, :]
                    if last and half == 1:
                        nc.vector.tensor_scalar_mul(dst, po3[:], 1.0 / 256.0)
                    else:
                        nc.scalar.mul(dst, po3[:], 1.0 / 256.0)
                    if last:
                        nc.sync.dma_start(
                            outC_d[st, tcc, :, 4 * half : 4 * half + 4, :], dst
                        )
                if not last:
                    nc.sync.dma_start(outC_d[st, tcc], o3sb[:, tcc, :, :])

            # ---- software-pipelined emission: chunks of st overlap the
            # projection of st+1 AND the output projection of st-1 ----
            FC_SCHED = [(0, 1, 2, 3), (4, 5), (6, 7), ()]
            from collections import deque
            pend = deque()  # out-projection chunks, emitted with lag 2
            ksbs = {}
            tiles = {0: emit_proj_tiles(0)}
            for fc in range(8):
                emit_fc(tiles[0], fc)
            for tcc in range(NCH):
                emit_v(tiles[0], tcc)
            alloc_ksb(ksbs, 0)
            for j in range(4):
                emit_ktr_j(tiles[0], ksbs, 0, j)
            prev = None  # (st-1, outT_sb, o3sb)
            for st in range(NST):
                T = tiles.pop(st)
                Tn = None
                if st + 1 < NST:
                    Tn = emit_proj_tiles(st + 1)
                    tiles[st + 1] = Tn
                    alloc_ksb(ksbs, st + 1)
                outT_sb = osp.tile(
                    [128, 4, ST], bf16, tag="outT_sb", name=f"osb{st}"
                )
                ohi = o8p.tile([128, 4, ST], fp8, tag="ohi", name=f"ohi{st}")
                olo = o8p.tile([128, 4, ST], fp8, tag="olo", name=f"olo{st}")
                o3sb = o3p.tile(
                    [128, NCH, 8, C], bf16, tag="o3sb", name=f"o3sb{st}"
                )
                for tcc in range(NCH):
                    sz8_prev = szh[0]
                    ATs = emit_chunk_a(T, st, tcc)
                    emit_chunk_b(
                        T, st, tcc, outT_sb, ohi, olo, ATs, sz8_prev,
                        ksbs.pop((st, tcc)),
                    )
                    pend.append((st, ohi, olo, o3sb, tcc))
                    lag = 4 if st < NST - 1 else 4
                    while len(pend) > lag:
                        emit_outproj_chunk(*pend.popleft())
                    if Tn is not None:
                        for fc in FC_SCHED[tcc]:
                            emit_fc(Tn, fc)
                            if fc >= 4:
                                emit_ktr_j(Tn, ksbs, st + 1, fc - 4)
                    if Tn is not None:
                        emit_v(Tn, tcc)
                prev = (st, ohi, olo, o3sb)
            while pend:
                emit_outproj_chunk(*pend.popleft())

    nc.finalize()
    return nc


def _get_nc():
    if "nc" not in _CACHE:
        _CACHE["nc"] = _build_nc()
    return _CACHE["nc"]


def _pack_x(xT):
    """xT: [1024, 4096] f32 -> hi/lo fp8 in [128, 4, 2, L] layout."""
    xr = xT.reshape(4, 2, 128, L).transpose(2, 0, 1, 3)  # [128, 4, 2, L]
    hi = xr.astype(F8)
    lo = (xr - hi.astype(np.float32)).astype(F8)
    return np.ascontiguousarray(hi), np.ascontiguousarray(lo)


def _make_in_maps(x, w_qkv, w_out):
    maskT = np.triu(np.ones((C, C), np.float32))  # maskT[j,i] = 1 if j <= i
    in_maps = []
    for core in range(8):
        b, g = core // 2, core % 2
        xT = np.ascontiguousarray(x[b].T).astype(np.float32)
        xhi, xlo = _pack_x(xT)
        # q/k weights x32 (fp8 subnormal avoidance), transposed
        W1 = np.concatenate(
            [
                w_qkv[512 * g : 512 * (g + 1)],
                w_qkv[1024 + 512 * g : 1024 + 512 * (g + 1)],
            ],
            axis=0,
        ).T.astype(np.float32)  # [1024 d, 1024 f]
        wqk = (32.0 * W1).reshape(4, 2, 128, 8, 128).transpose(2, 3, 0, 1, 4)
        wqk8 = np.ascontiguousarray(wqk).astype(F8)
        # v weights x32, hi/lo split
        Wv = (32.0 * w_qkv[2048 + 512 * g : 2048 + 512 * (g + 1)].T).astype(
            np.float32
        )  # [1024 d, 512 f]
        Wvr = Wv.reshape(4, 2, 128, FPC).transpose(2, 0, 1, 3)  # [128,4,2,512]
        wvh = Wvr.astype(F8)
        wvl = (Wvr - wvh.astype(np.float32)).astype(F8)
        # output projection weights [512 f, 1024 oc] -> fp8 hi/lo x32
        # layout [p, u, d, ocg, oc]: feature = 128*(2u+d)+p
        W2 = 32.0 * w_out[:, 512 * g : 512 * (g + 1)].T.astype(np.float32)
        w2r = W2.reshape(2, 2, 128, 8, 128).transpose(2, 0, 1, 3, 4)
        w28h = w2r.astype(F8)
        w28l = (w2r - w28h.astype(np.float32)).astype(F8)
        in_maps.append(
            {
                "xhi": xhi,
                "xlo": np.ascontiguousarray(xlo),
                "wqk": wqk8,
                "wvh": np.ascontiguousarray(wvh),
                "wvl": np.ascontiguousarray(wvl),
                "w28h": np.ascontiguousarray(w28h),
                "w28l": np.ascontiguousarray(w28l),
                "maskT": maskT,
            }
        )
    return in_maps


def _run(inputs, trace=False):
    from concourse.bass_utils import run_bass_kernel_spmd

    nc = _get_nc()
    in_maps = _make_in_maps(inputs["x"], inputs["w_qkv"], inputs["w_out"])
    res = run_bass_kernel_spmd(nc, in_maps, core_ids=list(range(8)), trace=trace)
    out = np.empty((B, L, D), np.float32)
    for b in range(B):
        # outC [st, c, p, g, t] -> [L, D]: token 512st+128c+t, feature 128g+p
        p0 = res.results[2 * b]["outC"].astype(np.float32)
        p1 = res.results[2 * b + 1]["outC"].astype(np.float32)
        out[b] = (p0 + p1).transpose(0, 1, 4, 3, 2).reshape(L, D)
    return out, res


def kernel(x, w_qkv, w_out):
    out, _ = _run({"x": x, "w_qkv": w_qkv, "w_out": w_out})
    return out


# revision 68
# speedup vs baseline: 1.0405x; 1.0405x over previous
"""TRN2 Bass kernel for nn_LinearAttention (B=4, L=4096, D=1024, H=16, dh=64).

Sharding: core c = batch c//2, heads (c%2)*8..+8. Zero cross-core comms; the
two half-head partial output projections per batch are summed on the host.

Per-core pipeline (super-tiles of 512 tokens, scan chunks of 128):
  1. QKV projection:
     - q/k: fp8e4 DoubleRow matmuls (0.5 cyc/row, K=256 per call). Weights are
       scaled x32 on the host (plain w_qkv values sit in e4m3's subnormal
       range and quantize at 6-50% error; x32 moves them to normals). A bias
       ones-row (weight 32) makes PSUM hold 32(z+1); phi is exact in 2 passes
       via 32*phi(z) = max(min(32*e^z, 32), 32z+32):
       Act Exp(scale=1/32, bias=ln32-1) then one DVE scalar_tensor_tensor.
       The x32 on q/k cancels between numerator and denominator.
     - v: fp8 DoubleRow, 2-term compensated (x_hi(w_hi+w_lo)); the very first
       chunk adds the x_lo w_hi term (early tokens average too few v's to
       absorb x-quantization noise). The copy to bf16 applies 1/128.
  2. Chunked linear attention, bf16 operands / f32 PSUM:
     - k chunks go token-major via XBAR DMA-transpose (sync queue, 16x128
       tiles at 14ns) - the PE never transposes.
     - A^T = K^T Q for all 8 heads into one 2-bank PSUM tile (parity-split
       tile rows) -> masked-copy per parity half (DVE, doubles as the
       PSUM->SBUF move) -> den matmuls first (rden overlaps O) -> O = A_m V +
       Q S8. den > 0 always, so the reference eps-clamp is skipped and
       rden = 1/den directly; every scale factor is folded into w2 on the
       host. State [S | z/32] lives PERSISTENTLY in one PSUM bank (start only
       at chunk 0) with a per-chunk bf16 snapshot for the next chunk's
       Q S / q.z reads.
  3. Normalized o (bf16, token-major) is XBAR DMA-transposed to feature-major;
     the idle Pool engine (all-SBUF, so legal there) derives an fp8 hi/lo
     split, and the output projection runs as 3-term compensated fp8
     DoubleRow (o_hi w_hi + o_hi w_lo + o_lo w_hi, weights x32 fp8 hi/lo on
     the host) at 0.75x the bf16 cost. bf16 partials are DMA'd out per chunk;
     the host upcasts and sums core pairs in f32.

Schedule: emission order is the engine execution order, so it is tuned as a
software pipeline: per chunk iteration of super-tile s we emit [attention
chunk] [output projection of the chunk 4 back] [2-4 q/k projections of s+1,
with k-transposes as each k group completes] [one v projection of s+1]. The
output projection lags 4 chunks so its fp8 operand chain (XBAR transpose ->
Pool hi/lo split) never blocks the PE. A junk-matmul warmup burst at t=0
ramps the PE p-state inside the initial DMA shadow. Engine placement:
PE matmuls only; Act: exp, v/sz8/o3sb PSUM->SBUF moves; DVE: phi combine,
mask-multiplies, reciprocal, o normalize; Pool: fp8 o split + weight DMAs;
sync: x loads, XBAR transposes, output stores. PSUM banks: 3 projection +
2 A^T + 1 O + 1 state + 1 den = 8.
"""
import sys

sys.path.insert(0, "/opt/trn_rl_repo")
import numpy as np
import ml_dtypes

F8 = ml_dtypes.float8_e4m3
BFNP = ml_dtypes.bfloat16

D = 1024
L = 4096
B = 4
H = 16
DH = 64
FPC = 512          # features per core (8 heads x 64)
C = 128            # scan chunk
ST = 512           # super-tile tokens
NCH = ST // C      # 4
NST = L // ST      # 8
EPS = 1e-6
V3TERM_ST = 1      # super-tiles using 3-term compensated v (rest use 2-term)

_CACHE = {}


def _build_nc():
    import concourse.bacc as bacc
    import concourse.mybir as mybir
    import concourse.tile as tile

    dt = mybir.dt
    f32, fp8, bf16 = dt.float32, dt.float8e4, dt.bfloat16
    Alu = mybir.AluOpType
    Act = mybir.ActivationFunctionType
    PM = mybir.MatmulPerfMode

    nc = bacc.Bacc("TRN2", target_bir_lowering=False, debug=True)

    xhi_d = nc.dram_tensor("xhi", [128, 4, 2, L], fp8, kind="ExternalInput")
    xlo_d = nc.dram_tensor("xlo", [128, 4, 2, L], fp8, kind="ExternalInput")
    wqk_d = nc.dram_tensor("wqk", [128, 8, 4, 2, 128], fp8, kind="ExternalInput")
    wvh_d = nc.dram_tensor("wvh", [128, 4, 2, FPC], fp8, kind="ExternalInput")
    wvl_d = nc.dram_tensor("wvl", [128, 4, 2, FPC], fp8, kind="ExternalInput")
    w28h_d = nc.dram_tensor("w28h", [128, 2, 2, 8, 128], fp8, kind="ExternalInput")
    w28l_d = nc.dram_tensor("w28l", [128, 2, 2, 8, 128], fp8, kind="ExternalInput")
    maskT_d = nc.dram_tensor("maskT", [C, C], f32, kind="ExternalInput")
    # [st, c, p, g, t]: output feature 128*g+p at token 512*st+128*c+t
    outC_d = nc.dram_tensor(
        "outC", [NST, NCH, 128, 8, C], bf16, kind="ExternalOutput"
    )

    from contextlib import ExitStack

    with tile.TileContext(nc) as tc, ExitStack() as es:
        if True:
            wp = es.enter_context(tc.tile_pool(name="wp", bufs=1))
            xp = es.enter_context(tc.tile_pool(name="xp", bufs=4))
            qp = es.enter_context(tc.tile_pool(name="qp", bufs=6))
            scr = es.enter_context(tc.tile_pool(name="scr", bufs=6))
            vp = es.enter_context(tc.tile_pool(name="vp", bufs=2))
            kp = es.enter_context(tc.tile_pool(name="kp", bufs=9))
            atsp = es.enter_context(tc.tile_pool(name="atsp", bufs=5))
            szsp = es.enter_context(tc.tile_pool(name="szsp", bufs=5))
            dp = es.enter_context(tc.tile_pool(name="dp", bufs=8))
            ocp = es.enter_context(tc.tile_pool(name="ocp", bufs=5))
            osp = es.enter_context(tc.tile_pool(name="osp", bufs=3))
            o8p = es.enter_context(tc.tile_pool(name="o8p", bufs=3))
            o3p = es.enter_context(tc.tile_pool(name="o3p", bufs=2))
            projp = es.enter_context(tc.tile_pool(name="projp", bufs=3, space="PSUM"))
            atp = es.enter_context(tc.tile_pool(name="atp", bufs=1, space="PSUM"))
            obp = es.enter_context(tc.tile_pool(name="obp", bufs=1, space="PSUM"))
            szp = es.enter_context(tc.tile_pool(name="szp", bufs=1, space="PSUM"))
            denp = es.enter_context(tc.tile_pool(name="denp", bufs=1, space="PSUM"))

            # ---- resident constants / weights (wqk split so fc0 can start
            # as soon as the first half lands) ----
            wqk4 = []
            for q4 in range(4):
                wq = wp.tile([128, 2, 4, 2, 128], fp8, tag=f"wqk{q4}")
                nc.scalar.dma_start(wq[:], wqk_d[:, 2 * q4 : 2 * q4 + 2])
                wqk4.append(wq)
            wvh = wp.tile([128, 4, 2, FPC], fp8, tag="wvh")
            nc.gpsimd.dma_start(wvh[:], wvh_d[:])
            wvl = wp.tile([128, 4, 2, FPC], fp8, tag="wvl")
            nc.gpsimd.dma_start(wvl[:], wvl_d[:])
            maskT = wp.tile([C, C], f32, tag="maskT")
            nc.gpsimd.dma_start(maskT[:], maskT_d[:])
            w28h = wp.tile([128, 2, 2, 8, 128], fp8, tag="w28h")
            w28l = wp.tile([128, 2, 2, 8, 128], fp8, tag="w28l")
            nc.gpsimd.dma_start(w28h[:], w28h_d[:])
            nc.gpsimd.dma_start(w28l[:], w28l_d[:])
            wb8 = wp.tile([128, 2, 128], fp8, tag="wb8")
            nc.vector.memset(wb8[:].bitcast(f32), 0.0)
            nc.vector.memset(wb8[0:1, 0, :], 32.0)
            xb8 = wp.tile([128, 2, ST], fp8, tag="xb8")
            nc.vector.memset(xb8[:].bitcast(f32), 0.0)
            nc.vector.memset(xb8[0:1, 0, :], 1.0)
            ones32 = wp.tile([128, 1], bf16, tag="ones32")
            nc.vector.memset(ones32[:], 1.0 / 32.0)
            bm1 = wp.tile([128, 1], f32, tag="bm1")
            nc.vector.memset(bm1[:], float(np.log(32.0) - 1.0))

            # PE clock-ramp warmup: the PE would idle ~3.5us waiting for the
            # first weight/x DMAs anyway; junk matmuls during that window ramp
            # the p-state so real work starts at full clock.
            warm = projp.tile([128, ST], f32, tag="proj", name="warmup")
            for w in range(8):
                nc.tensor.matmul(
                    warm[:],
                    wb8[:],
                    xb8[:, :, :],
                    start=(w == 0),
                    stop=(w == 7),
                    perf_mode=PM.DoubleRow,
                    skip_group_check=True,
                )

            # persistent scan state [S | z/32] per head pair, in PSUM.
            # Full-bank tile (2048B row pitch) so pending-zero bookkeeping
            # stays row-aligned for base_partition=64 matmul outputs.
            Sz_t = szp.tile([128, 512], f32, tag="Sz")
            Sz = Sz_t[:, 0:260].rearrange("p (j e) -> p j e", e=DH + 1)
            sz8_init = szsp.tile([128, 4, DH + 1], bf16, tag="Sz8", name="sz8i")
            nc.vector.memset(sz8_init[:], 0.0)
            szh = [sz8_init]  # rolling snapshot holder

            def emit_proj_tiles(st):
                t0 = st * ST
                xhi = xp.tile([128, 4, 2, ST], fp8, tag="xhi", name=f"xhi{st}")
                nc.sync.dma_start(xhi[:], xhi_d[:, :, :, t0 : t0 + ST])
                T = dict(xhi=xhi)
                if st < V3TERM_ST:
                    xlo = xp.tile([128, 4, 2, ST], fp8, tag="xlo", name=f"xlo{st}")
                    nc.sync.dma_start(xlo[:], xlo_d[:, :, :, t0 : t0 + ST])
                    T["xlo"] = xlo
                T["qTb"] = qp.tile([128, 4, ST], bf16, tag="qTb", name=f"qTb{st}")
                T["kTb"] = qp.tile([128, 4, ST], bf16, tag="kTb", name=f"kTb{st}")
                T["v1"] = vp.tile([128, NCH, 8, DH], bf16, tag="v1", name=f"v1{st}")
                return T

            def emit_fc(T, fc):
                pq = projp.tile([128, ST], f32, tag="proj", name=f"pq_{fc}")
                for hh in range(2):
                    sl = slice(256 * hh, 256 * hh + 256)
                    for j in range(4):
                        nc.tensor.matmul(
                            pq[:, sl],
                            wqk4[fc // 2][:, fc % 2, j, :, :],
                            T["xhi"][:, j, :, sl],
                            start=(hh == 0 and j == 0),
                            stop=False,
                            perf_mode=PM.DoubleRow,
                            skip_group_check=True,
                        )
                    nc.tensor.matmul(
                        pq[:, sl],
                        wb8[:],
                        xb8[:, :, sl],
                        start=False,
                        stop=(hh == 1),
                        perf_mode=PM.DoubleRow,
                        skip_group_check=True,
                    )
                texp = scr.tile([128, ST], f32, tag="texp", name=f"texp{fc}")
                nc.scalar.activation(
                    texp[:], pq[:], Act.Exp, bias=bm1[:], scale=1.0 / 32.0
                )
                dst = T["qTb"][:, fc, :] if fc < 4 else T["kTb"][:, fc - 4, :]
                nc.vector.scalar_tensor_tensor(
                    dst, texp[:], 32.0, pq[:], Alu.min, Alu.max
                )

            def emit_v(T, tcc):
                pv = projp.tile([128, FPC], f32, tag="proj", name=f"pv{tcc}")
                xc_hi = T["xhi"][:, :, :, tcc * 128 : (tcc + 1) * 128]
                if "xlo" in T and tcc == 0:
                    xc_lo = T["xlo"][:, :, :, tcc * 128 : (tcc + 1) * 128]
                    terms = ((xc_hi, wvh), (xc_hi, wvl), (xc_lo, wvh))
                else:
                    terms = ((xc_hi, wvh), (xc_hi, wvl))
                ncalls = 4 * len(terms)
                n = 0
                for hh in range(2):
                    sl = slice(256 * hh, 256 * hh + 256)
                    for xa, wv in terms:
                        for j in range(4):
                            n += 1
                            nc.tensor.matmul(
                                pv[:, sl],
                                xa[:, j, :, :],
                                wv[:, j, :, sl],
                                start=(n == 1 or n == ncalls + 1),
                                stop=(n == ncalls or n == 2 * ncalls),
                                perf_mode=PM.DoubleRow,
                                skip_group_check=True,
                            )
                nc.scalar.mul(
                    T["v1"][:, tcc, :, :],
                    pv[:].rearrange("p (h e) -> p h e", e=DH),
                    1.0 / 128.0,
                )

            def alloc_ksb(ksbs, st):
                for tcc in range(NCH):
                    ksbs[(st, tcc)] = kp.tile(
                        [128, 8, DH], bf16, tag="ksb", name=f"ksb{st * NCH + tcc}"
                    )

            def emit_ktr_j(T, ksbs, st, j):
                # XBAR DMA-transpose k group j to token-major for every chunk:
                # [128 feats, 128 toks] -> ksb[:, 2j:2j+2, :] (= heads 2j,2j+1).
                for tcc in range(NCH):
                    tsl = slice(tcc * 128, (tcc + 1) * 128)
                    nc.sync.dma_start_transpose(
                        ksbs[(st, tcc)][:, 2 * j : 2 * j + 2, :],
                        T["kTb"][:, j, tsl],
                    )

            def emit_chunk_a(T, st, tcc):
                # A^T + masked-copy; the PE cover work for the DVE mask is
                # emitted by the driver between chunk_a and chunk_b.
                cg = st * NCH + tcc
                tsl = slice(tcc * 128, (tcc + 1) * 128)
                qTb, kTb, v1 = T["qTb"], T["kTb"], T["v1"]

                # -- A^T = K^T Q; bank g = heads of parity g (a PE tile-row
                # config may not change within a PSUM bank); one 2-bank tile
                # so the masked-copy is a single DVE op --
                ATp = atp.tile([128, 2, 4, C], f32, tag="ATp", name=f"atp{cg}")
                for h in range(8):
                    po = 64 * (h % 2)
                    fq = h // 2
                    nc.tensor.matmul(
                        ATp[:, h % 2, h // 2, :],
                        kTb[po : po + 64, fq, tsl],
                        qTb[po : po + 64, fq, tsl],
                        start=(h <= 1),
                        stop=(h >= 6),
                        skip_group_check=True,
                    )
                ATs = atsp.tile([128, 8, C], bf16, tag="ATs", name=f"ats{cg}")
                for g in range(2):
                    nc.vector.tensor_tensor(
                        ATs[:, 4 * g : 4 * g + 4, :],
                        ATp[:, g],
                        maskT[:].unsqueeze(1).broadcast_to([C, 4, C]),
                        Alu.mult,
                    )

                return ATs

            def emit_chunk_b(T, st, tcc, outT_sb, ohi, olo, ATs, sz8_prev, ksb):
                cg = st * NCH + tcc
                tsl = slice(tcc * 128, (tcc + 1) * 128)
                qTb, v1 = T["qTb"], T["v1"]
                den = denp.tile([128, 8], f32, tag="den", name=f"den{cg}")

                HORD = (0, 2, 4, 6, 1, 3, 5, 7)  # even parity first: its
                # masked half lands first, so den/O can start sooner

                # -- den = A_m^T ones/32 + q . z8 (before O: rden overlaps O) --
                for i, h in enumerate(HORD):
                    po = 64 * (h % 2)
                    fq = h // 2
                    nc.tensor.matmul(
                        den[:, h : h + 1],
                        ATs[:, 4 * (h % 2) + h // 2, :],
                        ones32[:],
                        start=(i == 0),
                        stop=False,
                        skip_group_check=True,
                    )
                    nc.tensor.matmul(
                        den[:, h : h + 1],
                        qTb[po : po + 64, fq, tsl],
                        sz8_prev[po : po + 64, h // 2, DH : DH + 1],
                        start=False,
                        stop=(i == 7),
                        skip_group_check=True,
                    )
                # den > 0 always (phi > 0), so the reference eps-clamp is a
                # numeric no-op; the *8 scale restore is folded into w2.
                rden = dp.tile([C, 8], f32, tag="rden", name=f"rden{cg}")
                nc.vector.reciprocal(rden[:], den[:])

                # -- O = A_m V + Q S8 (one bank, 8 heads) --
                Ob = obp.tile([128, 8, DH], f32, tag="Ob", name=f"ob{cg}")
                for i, h in enumerate(HORD):
                    po = 64 * (h % 2)
                    fq = h // 2
                    nc.tensor.matmul(
                        Ob[:, h, :],
                        ATs[:, 4 * (h % 2) + h // 2, :],
                        v1[:, tcc, h, :],
                        start=(i == 0),
                        stop=False,
                        skip_group_check=True,
                    )
                    nc.tensor.matmul(
                        Ob[:, h, :],
                        qTb[po : po + 64, fq, tsl],
                        sz8_prev[po : po + 64, h // 2, 0:DH],
                        start=False,
                        stop=(i == 7),
                        skip_group_check=True,
                    )

                # -- state update into the persistent Sz bank --
                for h in range(8):
                    po = 64 * (h % 2)
                    j = h // 2
                    nc.tensor.matmul(
                        Sz[po : po + 64, j, 0:DH],
                        ksb[:, h, :],
                        v1[:, tcc, h, :],
                        start=(cg == 0 and h <= 1),
                        stop=False,
                        skip_group_check=True,
                    )
                    nc.tensor.matmul(
                        Sz[po : po + 64, j, DH : DH + 1],
                        ksb[:, h, :],
                        ones32[:],
                        start=False,
                        stop=(cg == NST * NCH - 1 and h == 7),
                        skip_group_check=True,
                    )
                sz8_new = szsp.tile(
                    [128, 4, DH + 1], bf16, tag="Sz8", name=f"sz8_{cg}"
                )
                nc.scalar.copy(sz8_new[:], Sz[:])
                szh[0] = sz8_new

                # -- normalize -> bf16 o_c (token-major) --
                oc8 = ocp.tile([C, 8, DH], bf16, tag="oc8", name=f"oc8{cg}")
                nc.vector.tensor_tensor(
                    oc8[:],
                    Ob[:],
                    rden[:].unsqueeze(2).broadcast_to([C, 8, DH]),
                    Alu.mult,
                )

                # -- XBAR DMA-transpose o_c to feature-major --
                oc_flat = oc8[:].rearrange("p h e -> p (h e)")
                for g in range(4):
                    nc.sync.dma_start_transpose(
                        outT_sb[:, g, tsl], oc_flat[:, g * 128 : (g + 1) * 128]
                    )
                # fp8 hi/lo split of feature-major o, on the (idle) Pool
                # engine: all-SBUF so it is legal there
                nc.gpsimd.tensor_copy(ohi[:, :, tsl], outT_sb[:, :, tsl])
                nc.gpsimd.tensor_tensor(
                    olo[:, :, tsl],
                    outT_sb[:, :, tsl],
                    ohi[:, :, tsl],
                    Alu.subtract,
                )

            def emit_outproj_chunk(st, ohi, olo, o3sb, tcc):
                # project one 128-token chunk to all 1024 output features:
                # 3-term compensated fp8 DoubleRow (o_hi w_hi + o_hi w_lo +
                # o_lo w_hi), then store (>=1KB contiguous runs both sides)
                tsl = slice(tcc * 128, (tcc + 1) * 128)
                last = st == NST - 1 and tcc == NCH - 1
                for half in range(2):
                    po3 = projp.tile(
                        [128, 4, C], f32, tag="proj", name=f"po3_{st}_{tcc}_{half}"
                    )
                    for g4 in range(4):
                        oc = 4 * half + g4
                        n = 0
                        for oa, wv in ((ohi, w28h), (ohi, w28l), (olo, w28h)):
                            for u in range(2):
                                n += 1
                                nc.tensor.matmul(
                                    po3[:, g4, :],
                                    wv[:, u, :, oc, :],
                                    oa[:, 2 * u : 2 * u + 2, tsl],
                                    start=(n == 1),
                                    stop=(n == 6),
                                    perf_mode=PM.DoubleRow,
                                    skip_group_check=True,
                                )
                    dst = o3sb[:, tcc, 4 * half : 4 * half + 4, :]
                    if last and half == 1:
                        nc.vector.tensor_scalar_mul(dst, po3[:], 1.0 / 256.0)
                    else:
                        nc.scalar.mul(dst, po3[:], 1.0 / 256.0)
                    if last:
                        nc.sync.dma_start(
                            outC_d[st, tcc, :, 4 * half : 4 * half + 4, :], dst
                        )
                if not last:
                    nc.sync.dma_start(outC_d[st, tcc], o3sb[:, tcc, :, :])

            # ---- software-pipelined emission: chunks of st overlap the
            # projection of st+1 AND the output projection of st-1 ----
            FC_SCHED = [(0, 1, 2, 3), (4, 5), (6, 7), ()]
            from collections import deque
            pend = deque()  # out-projection chunks, emitted with lag 2
            ksbs = {}
            tiles = {0: emit_proj_tiles(0)}
            for fc in range(8):
                emit_fc(tiles[0], fc)
            for tcc in range(NCH):
                emit_v(tiles[0], tcc)
            alloc_ksb(ksbs, 0)
            for j in range(4):
                emit_ktr_j(tiles[0], ksbs, 0, j)
            prev = None  # (st-1, outT_sb, o3sb)
            for st in range(NST):
                T = tiles.pop(st)
                Tn = None
                if st + 1 < NST:
                    Tn = emit_proj_tiles(st + 1)
                    tiles[st + 1] = Tn
                    alloc_ksb(ksbs, st + 1)
                outT_sb = osp.tile(
                    [128, 4, ST], bf16, tag="outT_sb", name=f"osb{st}"
                )
                ohi = o8p.tile([128, 4, ST], fp8, tag="ohi", name=f"ohi{st}")
                olo = o8p.tile([128, 4, ST], fp8, tag="olo", name=f"olo{st}")
                o3sb = o3p.tile(
                    [128, NCH, 8, C], bf16, tag="o3sb", name=f"o3sb{st}"
                )
                for tcc in range(NCH):
                    sz8_prev = szh[0]
                    ATs = emit_chunk_a(T, st, tcc)
                    emit_chunk_b(
                        T, st, tcc, outT_sb, ohi, olo, ATs, sz8_prev,
                        ksbs.pop((st, tcc)),
                    )
                    pend.append((st, ohi, olo, o3sb, tcc))
                    lag = 4 if st < NST - 1 else 4
                    while len(pend) > lag:
                        emit_outproj_chunk(*pend.popleft())
                    if Tn is not None:
                        for fc in FC_SCHED[tcc]:
                            emit_fc(Tn, fc)
                            if fc >= 4:
                                emit_ktr_j(Tn, ksbs, st + 1, fc - 4)
                    if Tn is not None:
                        emit_v(Tn, tcc)
                prev = (st, ohi, olo, o3sb)
            while pend:
                emit_outproj_chunk(*pend.popleft())

    nc.finalize()
    return nc


def _get_nc():
    if "nc" not in _CACHE:
        _CACHE["nc"] = _build_nc()
    return _CACHE["nc"]


def _pack_x(xT):
    """xT: [1024, 4096] f32 -> hi/lo fp8 in [128, 4, 2, L] layout."""
    xr = xT.reshape(4, 2, 128, L).transpose(2, 0, 1, 3)  # [128, 4, 2, L]
    hi = xr.astype(F8)
    lo = (xr - hi.astype(np.float32)).astype(F8)
    return np.ascontiguousarray(hi), np.ascontiguousarray(lo)


def _make_in_maps(x, w_qkv, w_out):
    maskT = np.triu(np.ones((C, C), np.float32))  # maskT[j,i] = 1 if j <= i
    in_maps = []
    for core in range(8):
        b, g = core // 2, core % 2
        xT = np.ascontiguousarray(x[b].T).astype(np.float32)
        xhi, xlo = _pack_x(xT)
        # q/k weights x32 (fp8 subnormal avoidance), transposed
        W1 = np.concatenate(
            [
                w_qkv[512 * g : 512 * (g + 1)],
                w_qkv[1024 + 512 * g : 1024 + 512 * (g + 1)],
            ],
            axis=0,
        ).T.astype(np.float32)  # [1024 d, 1024 f]
        wqk = (32.0 * W1).reshape(4, 2, 128, 8, 128).transpose(2, 3, 0, 1, 4)
        wqk8 = np.ascontiguousarray(wqk).astype(F8)
        # v weights x32, hi/lo split
        Wv = (32.0 * w_qkv[2048 + 512 * g : 2048 + 512 * (g + 1)].T).astype(
            np.float32
        )  # [1024 d, 512 f]
        Wvr = Wv.reshape(4, 2, 128, FPC).transpose(2, 0, 1, 3)  # [128,4,2,512]
        wvh = Wvr.astype(F8)
        wvl = (Wvr - wvh.astype(np.float32)).astype(F8)
        # output projection weights [512 f, 1024 oc] -> fp8 hi/lo x32
        # layout [p, u, d, ocg, oc]: feature = 128*(2u+d)+p
        W2 = 32.0 * w_out[:, 512 * g : 512 * (g + 1)].T.astype(np.float32)
        w2r = W2.reshape(2, 2, 128, 8, 128).transpose(2, 0, 1, 3, 4)
        w28h = w2r.astype(F8)
        w28l = (w2r - w28h.astype(np.float32)).astype(F8)
        in_maps.append(
            {
                "xhi": xhi,
                "xlo": np.ascontiguousarray(xlo),
                "wqk": wqk8,
                "wvh": np.ascontiguousarray(wvh),
                "wvl": np.ascontiguousarray(wvl),
                "w28h": np.ascontiguousarray(w28h),
                "w28l": np.ascontiguousarray(w28l),
                "maskT": maskT,
            }
        )
    return in_maps


def _run(inputs, trace=False):
    from concourse.bass_utils import run_bass_kernel_spmd

    nc = _get_nc()
    in_maps = _make_in_maps(inputs["x"], inputs["w_qkv"], inputs["w_out"])
    res = run_bass_kernel_spmd(nc, in_maps, core_ids=list(range(8)), trace=trace)
    out = np.empty((B, L, D), np.float32)
    for b in range(B):
        # outC [st, c, p, g, t] -> [L, D]: token 512st+128c+t, feature 128g+p
        p0 = res.results[2 * b]["outC"].astype(np.float32)
        p1 = res.results[2 * b + 1]["outC"].astype(np.float32)
        out[b] = (p0 + p1).transpose(0, 1, 4, 3, 2).reshape(L, D)
    return out, res


def kernel(x, w_qkv, w_out):
    out, _ = _run({"x": x, "w_qkv": w_qkv, "w_out": w_out})
    return out
